# revision 4
# baseline (speedup 1.0000x reference)
"""Distributed Trainium2 kernel for causal GQA attention with RoPE.

Tensor-parallel over heads across 8 NeuronCores: core c owns q heads
4c..4c+3 and kv head c.  Activations are kept transposed ([dim, seq])
so every matmul contracts over the partition axis:

  phase 1: qkvT = wqkv_c @ x.T   (RoPE fused into the PSUM eviction;
           v transposed back to [seq, dim] via PE transposes)
  phase 2: per head, causal flash attention, fully SBUF-resident:
           S = qT.T @ kT chunks -> masked exp (scale folded, row-sum
           accumulated) -> normalize -> PE-transpose P -> PV
  phase 3: AllGather oT over cores (partition-axis concat == o_proj
           contraction order), then out_cT = wo_c.T.T @ attn_allT.

Output per core is the transposed column shard [512, 2048] of the final
projection; the host concatenates and transposes.
"""

import math
import sys

if "/opt/trn_rl_repo" not in sys.path:
    sys.path.insert(0, "/opt/trn_rl_repo")

from contextlib import ExitStack

import numpy as np
import ml_dtypes

import concourse.bacc as bacc
import concourse.mybir as mybir
from concourse.tile import TileContext
from concourse.masks import make_identity, make_causal_mask
from concourse.bass_utils import run_bass_kernel_spmd

N_CORES = 8
H = 4096          # model dim
HD = 128          # head dim
QH = 4            # q heads per core
SCALE = 1.0 / math.sqrt(HD)
MASK_VAL = -1.0e5
DT = mybir.dt


def build_nc(S=2048):
    KT = H // 128           # contraction tiles for both GEMMs
    NCH = max(1, S // 512)  # seq chunks of 512
    CH = S // NCH           # chunk size
    QT = S // 128           # q row tiles
    M1 = QH + 2             # phase-1 output row tiles: 4 q heads, k, v

    nc = bacc.Bacc("TRN2", target_bir_lowering=False, debug=False,
                   num_devices=N_CORES)
    xT = nc.declare_dram_parameter("xT", [H, S], DT.bfloat16, isOutput=False)
    wqkvT = nc.declare_dram_parameter("wqkvT", [H, 128 * M1], DT.bfloat16,
                                      isOutput=False)
    woT = nc.declare_dram_parameter("woT", [H, 128 * QH], DT.bfloat16,
                                    isOutput=False)
    cosT = nc.declare_dram_parameter("cosT", [HD, S], DT.float32,
                                     isOutput=False)
    sinTs = nc.declare_dram_parameter("sinTs", [HD, S], DT.float32,
                                      isOutput=False)
    out = nc.declare_dram_parameter("out", [128 * QH, S], DT.float32,
                                    isOutput=True)


    with TileContext(nc) as tc, ExitStack() as ctx:
        persist = ctx.enter_context(tc.tile_pool(name="persist", bufs=1))
        cos_sb = persist.tile([HD, S], DT.float32, name="cos_sb")
        sins_sb = persist.tile([HD, S], DT.float32, name="sins_sb")
        nc.sync.dma_start(out=cos_sb[:], in_=cosT[:])
        nc.sync.dma_start(out=sins_sb[:], in_=sinTs[:])
        # qk_sb[0:4] = roped qT per head, qk_sb[4] = roped kT
        qk_sb = [persist.tile([128, S], DT.bfloat16, name=f"qk{m}")
                 for m in range(QH + 1)]
        vT_sb = persist.tile([128, S], DT.bfloat16, name="vT_sb")
        v_sb = persist.tile([128, S], DT.bfloat16, name="v_sb")
        oT_sb = [persist.tile([128, S], DT.bfloat16, name=f"oT{h}")
                 for h in range(QH)]
        ident = persist.tile([128, 128], DT.bfloat16, name="ident")
        make_identity(nc, ident[:])
        maskb = persist.tile([128, 128], DT.float32, name="maskb")
        make_causal_mask(nc, maskb[:], mask_val=MASK_VAL)

        # ---------------- phase 1: qkv projection + rope -----------------
        with ExitStack() as s1:
            wq_pool = s1.enter_context(tc.tile_pool(name="wqkv", bufs=1))
            wq_tiles = []
            for k in range(KT):
                t = wq_pool.tile([128, 128 * M1], DT.bfloat16,
                                 name=f"wqkv_k{k}", tag=f"wqkv{k}")
                nc.sync.dma_start(out=t[:], in_=wqkvT[128 * k:128 * (k + 1), :])
                wq_tiles.append(t)
            xpool = s1.enter_context(tc.tile_pool(name="xpool", bufs=4))
            acc1 = s1.enter_context(tc.tile_pool(name="acc1", bufs=1,
                                                 space="PSUM"))
            tmp1 = s1.enter_context(tc.tile_pool(name="tmp1", bufs=2))
            for cb in range(NCH):
                ns = slice(CH * cb, CH * (cb + 1))
                accs = [acc1.tile([128, CH], DT.float32,
                                  name=f"acc1_{cb}_{m}", tag=f"acc{m}")
                        for m in range(M1)]
                for k in range(KT):
                    xt = xpool.tile([128, CH], DT.bfloat16,
                                    name=f"x_{cb}_{k}", tag="xt")
                    nc.sync.dma_start(out=xt[:],
                                      in_=xT[128 * k:128 * (k + 1), ns])
                    for m in range(M1):
                        nc.tensor.matmul(accs[m][:],
                                         wq_tiles[k][:, 128 * m:128 * (m + 1)],
                                         xt[:],
                                         start=(k == 0), stop=(k == KT - 1))
                for m in range(QH + 1):
                    # rope: out = acc*cos + swap_halves(acc)*sin_signed
                    tmp = tmp1.tile([128, CH], DT.float32,
                                    name=f"tmp_{cb}_{m}", tag="tmp")
                    nc.vector.tensor_tensor(out=tmp[0:64, :],
                                            in0=accs[m][64:128, :],
                                            in1=sins_sb[0:64, ns],
                                            op=mybir.AluOpType.mult)
                    nc.vector.tensor_tensor(out=tmp[64:128, :],
                                            in0=accs[m][0:64, :],
                                            in1=sins_sb[64:128, ns],
                                            op=mybir.AluOpType.mult)
                    nc.vector.tensor_tensor(out=qk_sb[m][:, ns],
                                            in0=accs[m][:],
                                            in1=cos_sb[:, ns],
                                            op=mybir.AluOpType.mult)
                    nc.vector.tensor_tensor(out=qk_sb[m][:, ns],
                                            in0=qk_sb[m][:, ns],
                                            in1=tmp[:],
                                            op=mybir.AluOpType.add)
                nc.scalar.copy(vT_sb[:, ns], accs[QH + 1][:])
            # transpose v back to [seq, dim]
            vtp = s1.enter_context(tc.tile_pool(name="vtp", bufs=2,
                                                space="PSUM"))
            for j in range(QT):
                js = slice(128 * j, 128 * (j + 1))
                t = vtp.tile([128, 128], DT.bfloat16, name=f"vtp{j}",
                             tag="vtp")
                nc.tensor.transpose(t[:], vT_sb[:, js], ident[:])
                nc.scalar.copy(v_sb[:, js], t[:])

        # ---------------- phase 2: causal attention per head -------------
        with ExitStack() as s2:
            sc_pool = s2.enter_context(tc.tile_pool(name="scp", bufs=1,
                                                    space="PSUM"))
            tp_pool = s2.enter_context(tc.tile_pool(name="tpp", bufs=2,
                                                    space="PSUM"))
            ot_pool = s2.enter_context(tc.tile_pool(name="otp", bufs=2,
                                                    space="PSUM"))
            p_pool = s2.enter_context(tc.tile_pool(name="pp", bufs=2))
            pt_pool = s2.enter_context(tc.tile_pool(name="ptp", bufs=3))
            rs_pool = s2.enter_context(tc.tile_pool(name="rsp", bufs=2))
            for h in range(QH):
                for qi in range(QT):
                    sk_end = 128 * (qi + 1)
                    qs = slice(128 * qi, 128 * (qi + 1))
                    sc = sc_pool.tile([128, sk_end], DT.float32,
                                      name=f"sc{h}_{qi}", tag="sc")
                    for c0 in range(0, sk_end, 512):
                        cs = slice(c0, min(c0 + 512, sk_end))
                        nc.tensor.matmul(sc[:, cs],
                                         qk_sb[h][:, qs],
                                         qk_sb[QH][:, cs],
                                         start=True, stop=True)
                    nc.vector.tensor_tensor(out=sc[:, 128 * qi:sk_end],
                                            in0=sc[:, 128 * qi:sk_end],
                                            in1=maskb[:],
                                            op=mybir.AluOpType.add)
                    P = p_pool.tile([128, sk_end], DT.bfloat16,
                                    name=f"P{h}_{qi}", tag="P")
                    rs = rs_pool.tile([128, 1], DT.float32,
                                      name=f"rs{h}_{qi}", tag="rs")
                    nc.scalar.activation(P[:], sc[:],
                                         mybir.ActivationFunctionType.Exp,
                                         bias=0.0, scale=SCALE,
                                         accum_out=rs[:])
                    rcp = rs_pool.tile([128, 1], DT.float32,
                                       name=f"rcp{h}_{qi}", tag="rcp")
                    nc.vector.reciprocal(rcp[:], rs[:])
                    nc.vector.tensor_scalar(out=P[:], in0=P[:],
                                            scalar1=rcp[:], scalar2=None,
                                            op0=mybir.AluOpType.mult)
                    ot = ot_pool.tile([128, 128], DT.float32,
                                      name=f"ot{h}_{qi}", tag="ot")
                    for j in range(qi + 1):
                        js = slice(128 * j, 128 * (j + 1))
                        ptp = tp_pool.tile([128, 128], DT.bfloat16,
                                           name=f"ptp{h}_{qi}_{j}", tag="ptp")
                        nc.tensor.transpose(ptp[:], P[:, js], ident[:])
                        pts = pt_pool.tile([128, 128], DT.bfloat16,
                                           name=f"pts{h}_{qi}_{j}", tag="pts")
                        nc.any.tensor_copy(pts[:], ptp[:])
                        nc.tensor.matmul(ot[:], v_sb[:, js], pts[:],
                                         start=(j == 0), stop=(j == qi))
                    nc.any.tensor_copy(oT_sb[h][:, qs], ot[:])

        # ---------------- phase 3: allgather + output projection ---------
        with ExitStack() as s3:
            dpool = s3.enter_context(tc.tile_pool(name="dramp", bufs=1,
                                                  space="DRAM"))
            ag_in = dpool.tile([128 * QH, S], DT.bfloat16, name="ag_in")
            ag_out = dpool.tile([128 * QH * N_CORES, S], DT.bfloat16,
                                name="ag_out", addr_space="Shared")
            for h in range(QH):
                nc.sync.dma_start(out=ag_in[128 * h:128 * (h + 1), :],
                                  in_=oT_sb[h][:])
            nc.gpsimd.collective_compute(
                "AllGather", mybir.AluOpType.bypass,
                replica_groups=[list(range(N_CORES))],
                ins=[ag_in[:]], outs=[ag_out[:]])
            wo_pool = s3.enter_context(tc.tile_pool(name="wop", bufs=4))
            agp = s3.enter_context(tc.tile_pool(name="agp", bufs=4))
            acc3 = s3.enter_context(tc.tile_pool(name="acc3", bufs=1,
                                                 space="PSUM"))
            osb = s3.enter_context(tc.tile_pool(name="osb", bufs=2))
            KT3 = (128 * QH * N_CORES) // 128
            for cb in range(NCH):
                ns = slice(CH * cb, CH * (cb + 1))
                accs = [acc3.tile([128, CH], DT.float32,
                                  name=f"acc3_{cb}_{m}", tag=f"a3_{m}")
                        for m in range(QH)]
                for k in range(KT3):
                    wt = wo_pool.tile([128, 128 * QH], DT.bfloat16,
                                      name=f"wo_{cb}_{k}", tag="wo")
                    nc.sync.dma_start(out=wt[:],
                                      in_=woT[128 * k:128 * (k + 1), :])
                    at = agp.tile([128, CH], DT.bfloat16,
                                  name=f"ag_{cb}_{k}", tag="ag")
                    nc.sync.dma_start(out=at[:],
                                      in_=ag_out[128 * k:128 * (k + 1), ns])
                    for m in range(QH):
                        nc.tensor.matmul(accs[m][:],
                                         wt[:, 128 * m:128 * (m + 1)], at[:],
                                         start=(k == 0), stop=(k == KT3 - 1))
                for m in range(QH):
                    ob = osb.tile([128, CH], DT.float32,
                                  name=f"o3_{cb}_{m}", tag="o3")
                    nc.scalar.copy(ob[:], accs[m][:])
                    nc.sync.dma_start(out=out[128 * m:128 * (m + 1), ns],
                                      in_=ob[:])

    nc.compile()
    return nc


def host_inputs(x, wq, wk, wv, wo, S=2048):
    """Shard + preprocess full inputs into per-core input maps."""
    bf16 = ml_dtypes.bfloat16
    xT = np.ascontiguousarray(x.reshape(S, H).T).astype(bf16)
    inv_freq = 1.0 / (500000.0 ** (np.arange(0, HD, 2, dtype=np.float32) / HD))
    t = np.arange(S, dtype=np.float32)
    emb = np.concatenate([np.outer(t, inv_freq)] * 2, axis=-1)  # [S, HD]
    cosT = np.ascontiguousarray(np.cos(emb).T).astype(np.float32)
    sinT = np.cos(0)  # placeholder, replaced below
    sinT = np.ascontiguousarray(np.sin(emb).T).astype(np.float32)
    sinTs = sinT.copy()
    sinTs[0:64] = -sinTs[0:64]  # sign-folded for the rotate_half add
    in_maps = []
    for c in range(N_CORES):
        wqkv = np.concatenate([
            wq[128 * QH * c:128 * QH * (c + 1)],
            wk[HD * c:HD * (c + 1)],
            wv[HD * c:HD * (c + 1)],
        ], axis=0)  # [768, H]
        wqkvT = np.ascontiguousarray(wqkv.T).astype(bf16)
        woT = np.ascontiguousarray(
            wo[128 * QH * c:128 * QH * (c + 1)].T).astype(bf16)
        in_maps.append({
            "xT": xT, "wqkvT": wqkvT, "woT": woT,
            "cosT": cosT, "sinTs": sinTs,
        })
    return in_maps


_NC_CACHE = {}


def _get_nc(S=2048):
    if S not in _NC_CACHE:
        _NC_CACHE[S] = build_nc(S)
    return _NC_CACHE[S]


def run(inputs, S=2048, trace=False):
    nc = _get_nc(S)
    in_maps = host_inputs(inputs["x"], inputs["wq"], inputs["wk"],
                          inputs["wv"], inputs["wo"], S=S)
    res = run_bass_kernel_spmd(nc, in_maps, list(range(N_CORES)),
                               trace=trace)
    outp = np.empty((1, S, H), dtype=np.float32)
    for c in range(N_CORES):
        outp[0, :, 128 * QH * c:128 * QH * (c + 1)] = res.results[c]["out"].T
    return outp, res


def kernel(**inputs):
    outp, _ = run(inputs, S=2048, trace=False)
    return outp


# revision 9
# speedup vs baseline: 1.3279x; 1.3279x over previous
"""Distributed Trainium2 kernel for causal GQA attention with RoPE.

Tensor-parallel over heads across 8 NeuronCores: core c owns q heads
4c..4c+3 and kv head c.  Activations are kept transposed ([dim, seq])
so every matmul contracts over the partition axis:

  phase 1: qkvT = wqkv_c @ x.T   (RoPE fused into the PSUM eviction;
           v transposed back to [seq, dim] via PE transposes)
  phase 2: per head, causal flash attention, fully SBUF-resident:
           S = qT.T @ kT chunks -> masked exp (scale folded, row-sum
           accumulated) -> normalize -> PE-transpose P -> PV
  phase 3: AllGather oT over cores (partition-axis concat == o_proj
           contraction order), then out_cT = wo_c.T.T @ attn_allT.

Output per core is the transposed column shard [512, 2048] of the final
projection; the host concatenates and transposes.
"""

import math
import sys

if "/opt/trn_rl_repo" not in sys.path:
    sys.path.insert(0, "/opt/trn_rl_repo")

from contextlib import ExitStack

import numpy as np
import ml_dtypes

import concourse.bacc as bacc
import concourse.mybir as mybir
from concourse.tile import TileContext
from concourse.masks import make_identity, make_causal_mask
from concourse.bass_utils import run_bass_kernel_spmd

N_CORES = 8
H = 4096          # model dim
HD = 128          # head dim
QH = 4            # q heads per core
SCALE = 1.0 / math.sqrt(HD)
MASK_VAL = -1.0e5
DT = mybir.dt


def build_nc(S=2048):
    KT = H // 128           # contraction tiles for both GEMMs
    NCH = max(1, S // 512)  # seq chunks of 512
    CH = S // NCH           # chunk size
    QT = S // 128           # q row tiles
    M1 = QH + 2             # phase-1 output row tiles: 4 q heads, k, v

    nc = bacc.Bacc("TRN2", target_bir_lowering=False, debug=False,
                   num_devices=N_CORES)
    xT = nc.declare_dram_parameter("xT", [H, S], DT.bfloat16, isOutput=False)
    wqkvT = nc.declare_dram_parameter("wqkvT", [H, 128 * M1], DT.bfloat16,
                                      isOutput=False)
    woT = nc.declare_dram_parameter("woT", [H, 128 * QH], DT.bfloat16,
                                    isOutput=False)
    cosT = nc.declare_dram_parameter("cosT", [HD, S], DT.float32,
                                     isOutput=False)
    sinTs = nc.declare_dram_parameter("sinTs", [HD, S], DT.float32,
                                      isOutput=False)
    out = nc.declare_dram_parameter("out", [128 * QH, S], DT.float32,
                                    isOutput=True)


    with TileContext(nc) as tc, ExitStack() as ctx:
        persist = ctx.enter_context(tc.tile_pool(name="persist", bufs=1))
        cos_sb = persist.tile([HD, S], DT.float32, name="cos_sb")
        sins_sb = persist.tile([HD, S], DT.float32, name="sins_sb")
        # qk_sb[0:4] = roped qT per head, qk_sb[4] = roped kT
        qk_sb = [persist.tile([128, S], DT.bfloat16, name=f"qk{m}")
                 for m in range(QH + 1)]
        vT_sb = persist.tile([128, S], DT.bfloat16, name="vT_sb")
        v_sb = persist.tile([128, S], DT.bfloat16, name="v_sb")
        oT_sb = [persist.tile([128, S], DT.bfloat16, name=f"oT{h}")
                 for h in range(QH)]
        ident = persist.tile([128, 128], DT.bfloat16, name="ident")
        make_identity(nc, ident[:])
        maskb = persist.tile([128, 128], DT.float32, name="maskb")
        make_causal_mask(nc, maskb[:], mask_val=MASK_VAL)

        # ---------------- phase 1: qkv projection + rope -----------------
        with ExitStack() as s1:
            wq_pool = s1.enter_context(tc.tile_pool(name="wqkv", bufs=1))
            wq_tiles = [wq_pool.tile([128, 128 * M1], DT.bfloat16,
                                     name=f"wqkv_k{k}", tag=f"wqkv{k}")
                        for k in range(KT)]
            xpool = s1.enter_context(tc.tile_pool(name="xpool", bufs=4))
            acc1 = s1.enter_context(tc.tile_pool(name="acc1", bufs=1,
                                                 space="PSUM"))
            tmp1 = s1.enter_context(tc.tile_pool(name="tmp1", bufs=2))
            for cb in range(NCH):
                ns = slice(CH * cb, CH * (cb + 1))
                accs = [acc1.tile([128, CH], DT.float32,
                                  name=f"acc1_{cb}_{m}", tag=f"acc{m}")
                        for m in range(M1)]
                for k in range(KT):
                    if cb == 0:
                        # lazy weight DMA keeps the first matmuls' inputs
                        # at the front of the DMA queues
                        nc.sync.dma_start(out=wq_tiles[k][:],
                                          in_=wqkvT[128 * k:128 * (k + 1), :])
                    xt = xpool.tile([128, CH], DT.bfloat16,
                                    name=f"x_{cb}_{k}", tag="xt")
                    nc.sync.dma_start(out=xt[:],
                                      in_=xT[128 * k:128 * (k + 1), ns])
                    if cb == 0 and k == 1:
                        nc.sync.dma_start(out=cos_sb[:], in_=cosT[:])
                        nc.sync.dma_start(out=sins_sb[:], in_=sinTs[:])
                    for m in range(M1):
                        nc.tensor.matmul(accs[m][:],
                                         wq_tiles[k][:, 128 * m:128 * (m + 1)],
                                         xt[:],
                                         start=(k == 0), stop=(k == KT - 1))
                for m in range(QH + 1):
                    # rope: out = acc*cos + swap_halves(acc)*sin_signed
                    tmp = tmp1.tile([128, CH], DT.float32,
                                    name=f"tmp_{cb}_{m}", tag="tmp")
                    nc.vector.tensor_tensor(out=tmp[0:64, :],
                                            in0=accs[m][64:128, :],
                                            in1=sins_sb[0:64, ns],
                                            op=mybir.AluOpType.mult)
                    nc.vector.tensor_tensor(out=tmp[64:128, :],
                                            in0=accs[m][0:64, :],
                                            in1=sins_sb[64:128, ns],
                                            op=mybir.AluOpType.mult)
                    nc.vector.tensor_tensor(out=qk_sb[m][:, ns],
                                            in0=accs[m][:],
                                            in1=cos_sb[:, ns],
                                            op=mybir.AluOpType.mult)
                    nc.vector.tensor_tensor(out=qk_sb[m][:, ns],
                                            in0=qk_sb[m][:, ns],
                                            in1=tmp[:],
                                            op=mybir.AluOpType.add)
                nc.scalar.copy(vT_sb[:, ns], accs[QH + 1][:])
            # transpose v back to [seq, dim]
            vtp = s1.enter_context(tc.tile_pool(name="vtp", bufs=2,
                                                space="PSUM"))
            for j in range(QT):
                js = slice(128 * j, 128 * (j + 1))
                t = vtp.tile([128, 128], DT.bfloat16, name=f"vtp{j}",
                             tag="vtp")
                nc.tensor.transpose(t[:], vT_sb[:, js], ident[:])
                nc.scalar.copy(v_sb[:, js], t[:])

        # ---------------- phase 2: causal attention per head -------------
        # q-tiles processed in groups of GQ so the PV matmuls run at
        # N = 128*GQ and the P transposes batch into shared PSUM tiles.
        GQ = min(4, QT)
        NG = QT // GQ
        dpool = ctx.enter_context(tc.tile_pool(name="dramp", bufs=1,
                                               space="DRAM"))
        ag_in = []
        ag_out = []
        for h in range(QH):
            gi = dpool.tile([128, S], DT.bfloat16, name=f"ag_in{h}")
            go = dpool.tile([128 * N_CORES, S], DT.bfloat16,
                            name=f"ag_out{h}", addr_space="Shared")
            ag_in.append(gi)
            ag_out.append(go)
        with ExitStack() as s2:
            sc_pool = s2.enter_context(tc.tile_pool(name="scp", bufs=4,
                                                    space="PSUM"))
            tp_pool = s2.enter_context(tc.tile_pool(name="tpp", bufs=2,
                                                    space="PSUM"))
            ot_pool = s2.enter_context(tc.tile_pool(name="otp", bufs=2,
                                                    space="PSUM"))
            p_pool = s2.enter_context(tc.tile_pool(name="pp", bufs=2))
            pt_pool = s2.enter_context(tc.tile_pool(name="ptsb", bufs=18))
            rs_pool = s2.enter_context(tc.tile_pool(name="rsp", bufs=2))
            for h in range(QH):
                for g in range(NG):
                    Ps = []
                    for t in range(GQ):
                        qi = GQ * g + t
                        sk_end = 128 * (qi + 1)
                        qs = slice(128 * qi, 128 * (qi + 1))
                        P = p_pool.tile([128, sk_end], DT.bfloat16,
                                        name=f"P{h}_{qi}", tag=f"Pt{t}")
                        partials = []
                        for c0 in range(0, sk_end, 512):
                            c1 = min(c0 + 512, sk_end)
                            sc = sc_pool.tile([128, c1 - c0], DT.float32,
                                              name=f"sc{h}_{qi}_{c0}",
                                              tag="sc")
                            nc.tensor.matmul(sc[:], qk_sb[h][:, qs],
                                             qk_sb[QH][:, c0:c1],
                                             start=True, stop=True)
                            if c1 == sk_end:
                                nc.vector.tensor_tensor(
                                    out=sc[:, c1 - c0 - 128:],
                                    in0=sc[:, c1 - c0 - 128:],
                                    in1=maskb[:],
                                    op=mybir.AluOpType.add)
                            rs = rs_pool.tile([128, 1], DT.float32,
                                              name=f"rs{h}_{qi}_{c0}",
                                              tag="rs")
                            nc.scalar.activation(
                                P[:, c0:c1], sc[:],
                                mybir.ActivationFunctionType.Exp,
                                bias=0.0, scale=SCALE, accum_out=rs[:])
                            partials.append(rs)
                        tot = partials[0]
                        for ci in range(1, len(partials)):
                            nt = rs_pool.tile([128, 1], DT.float32,
                                              name=f"rt{h}_{qi}_{ci}",
                                              tag="rt")
                            nc.vector.tensor_tensor(out=nt[:], in0=tot[:],
                                                    in1=partials[ci][:],
                                                    op=mybir.AluOpType.add)
                            tot = nt
                        rcp = rs_pool.tile([128, 1], DT.float32,
                                           name=f"rcp{h}_{qi}", tag="rcp")
                        nc.vector.reciprocal(rcp[:], tot[:])
                        nc.vector.tensor_scalar(out=P[:], in0=P[:],
                                                scalar1=rcp[:], scalar2=None,
                                                op0=mybir.AluOpType.mult)
                        Ps.append(P)
                    ot = ot_pool.tile([128, 128 * GQ], DT.float32,
                                      name=f"ot{h}_{g}", tag="ot")
                    jmax = GQ * g + GQ
                    # pass 1: all transposes + psum->sbuf copies; pass 2:
                    # all PV matmuls.  Keeps the PE from stalling on the
                    # copy of each j before its PV.
                    pts_tiles = []
                    for j in range(jmax):
                        js = slice(128 * j, 128 * (j + 1))
                        ptp = tp_pool.tile([128, 128 * GQ], DT.bfloat16,
                                           name=f"ptp{h}_{g}_{j}", tag="ptp")
                        d0 = max(0, j - GQ * g)  # tiles t < d0 fully masked
                        for t in range(d0, GQ):
                            nc.tensor.transpose(
                                ptp[:, 128 * t:128 * (t + 1)],
                                Ps[t][:, js], ident[:])
                        pts = pt_pool.tile([128, 128 * GQ], DT.bfloat16,
                                           name=f"pts{h}_{g}_{j}", tag="pts")
                        if d0 > 0:
                            nc.vector.memset(pts[:, 0:128 * d0], 0.0)
                            nc.any.tensor_copy(pts[:, 128 * d0:],
                                               ptp[:, 128 * d0:])
                        else:
                            nc.any.tensor_copy(pts[:], ptp[:])
                        pts_tiles.append(pts)
                    for j in range(jmax):
                        js = slice(128 * j, 128 * (j + 1))
                        nc.tensor.matmul(ot[:], v_sb[:, js], pts_tiles[j][:],
                                         start=(j == 0), stop=(j == jmax - 1))
                    nc.any.tensor_copy(
                        oT_sb[h][:, 128 * GQ * g:128 * GQ * (g + 1)], ot[:])
                # allgather this head while later heads still compute
                nc.sync.dma_start(out=ag_in[h][:], in_=oT_sb[h][:])
                nc.gpsimd.collective_compute(
                    "AllGather", mybir.AluOpType.bypass,
                    replica_groups=[list(range(N_CORES))],
                    ins=[ag_in[h][:]], outs=[ag_out[h][:]])

        # ---------------- phase 3: output projection ----------------------
        with ExitStack() as s3:
            wo_pool = s3.enter_context(tc.tile_pool(name="wop", bufs=4))
            agp = s3.enter_context(tc.tile_pool(name="agp", bufs=4))
            acc3 = s3.enter_context(tc.tile_pool(name="acc3", bufs=1,
                                                 space="PSUM"))
            osb = s3.enter_context(tc.tile_pool(name="osb", bufs=2))
            for cb in range(NCH):
                ns = slice(CH * cb, CH * (cb + 1))
                accs = [acc3.tile([128, CH], DT.float32,
                                  name=f"acc3_{cb}_{m}", tag=f"a3_{m}")
                        for m in range(QH)]
                ki = 0
                for h in range(QH):
                    for r in range(N_CORES):
                        kk = 128 * (QH * r + h)  # global contraction row
                        wt = wo_pool.tile([128, 128 * QH], DT.bfloat16,
                                          name=f"wo_{cb}_{h}_{r}", tag="wo")
                        nc.sync.dma_start(out=wt[:],
                                          in_=woT[kk:kk + 128, :])
                        at = agp.tile([128, CH], DT.bfloat16,
                                      name=f"ag_{cb}_{h}_{r}", tag="ag")
                        nc.sync.dma_start(
                            out=at[:],
                            in_=ag_out[h][128 * r:128 * (r + 1), ns])
                        for m in range(QH):
                            nc.tensor.matmul(accs[m][:],
                                             wt[:, 128 * m:128 * (m + 1)],
                                             at[:],
                                             start=(ki == 0),
                                             stop=(ki == QH * N_CORES - 1))
                        ki += 1
                for m in range(QH):
                    ob = osb.tile([128, CH], DT.float32,
                                  name=f"o3_{cb}_{m}", tag="o3")
                    nc.scalar.copy(ob[:], accs[m][:])
                    nc.sync.dma_start(out=out[128 * m:128 * (m + 1), ns],
                                      in_=ob[:])

    nc.compile()
    return nc


def host_inputs(x, wq, wk, wv, wo, S=2048):
    """Shard + preprocess full inputs into per-core input maps."""
    bf16 = ml_dtypes.bfloat16
    xT = np.ascontiguousarray(x.reshape(S, H).T).astype(bf16)
    inv_freq = 1.0 / (500000.0 ** (np.arange(0, HD, 2, dtype=np.float32) / HD))
    t = np.arange(S, dtype=np.float32)
    emb = np.concatenate([np.outer(t, inv_freq)] * 2, axis=-1)  # [S, HD]
    cosT = np.ascontiguousarray(np.cos(emb).T).astype(np.float32)
    sinT = np.cos(0)  # placeholder, replaced below
    sinT = np.ascontiguousarray(np.sin(emb).T).astype(np.float32)
    sinTs = sinT.copy()
    sinTs[0:64] = -sinTs[0:64]  # sign-folded for the rotate_half add
    in_maps = []
    for c in range(N_CORES):
        wqkv = np.concatenate([
            wq[128 * QH * c:128 * QH * (c + 1)],
            wk[HD * c:HD * (c + 1)],
            wv[HD * c:HD * (c + 1)],
        ], axis=0)  # [768, H]
        wqkvT = np.ascontiguousarray(wqkv.T).astype(bf16)
        woT = np.ascontiguousarray(
            wo[128 * QH * c:128 * QH * (c + 1)].T).astype(bf16)
        in_maps.append({
            "xT": xT, "wqkvT": wqkvT, "woT": woT,
            "cosT": cosT, "sinTs": sinTs,
        })
    return in_maps


_NC_CACHE = {}


def _get_nc(S=2048):
    if S not in _NC_CACHE:
        _NC_CACHE[S] = build_nc(S)
    return _NC_CACHE[S]


def run(inputs, S=2048, trace=False):
    nc = _get_nc(S)
    in_maps = host_inputs(inputs["x"], inputs["wq"], inputs["wk"],
                          inputs["wv"], inputs["wo"], S=S)
    res = run_bass_kernel_spmd(nc, in_maps, list(range(N_CORES)),
                               trace=trace)
    outp = np.empty((1, S, H), dtype=np.float32)
    for c in range(N_CORES):
        outp[0, :, 128 * QH * c:128 * QH * (c + 1)] = res.results[c]["out"].T
    return outp, res


def kernel(**inputs):
    outp, _ = run(inputs, S=2048, trace=False)
    return outp
